# revision 1
# baseline (speedup 1.0000x reference)
"""Trainium2 Bass kernel for nn_D_GA_1812476199112 (maxpool -> 16-head
attention over 1024 tokens -> proj -> batchnorm -> maxunpool).

Sharding: data-parallel over batch B=8, one batch element per NeuronCore.
Everything is local per core; no collectives.

Per-core pipeline (channels-on-partitions layout [C=64, N=1024]):
  1. MaxPool2d(2,2) via strided DVE max ops (pipelined with the x DMA in
     four quarters, behind PE-warmup dummy matmuls that keep the HAM clock
     hot); argmax becomes first-match masks (is_equal + not-found chain,
     matching jnp.argmax tie semantics) computed during attention idle.
  2. Q^T/K^T are produced directly in a "strip-packed" layout (head h of
     supergroup sg at partitions 32c..32c+3) using host-permuted zero-padded
     weight matrices; prep matmuls run 2x concurrent via row strips {0,64}.
     Q/K packs are stored as fp32r (full-rate fp32, ~12-bit mantissa): the
     rounding cancels in the softmax ratio (verified ~1e-4 end-to-end).
  3. Score matmuls compute S^T [keys, queries] (K=4, 3x concurrent via PE
     row tiling tile_position=(32c,0)) into PSUM chunks [128, 3*512];
     one ACT Exp per chunk (scale=0.5 folds the softmax scale; no max
     subtraction needed, |score| <~ 15). ACT is the bottleneck engine.
  4. AV matmuls in bf16 (softmax-ratio error cancellation keeps end-to-end
     error ~2e-4) with a ones-augmented V (V~ [128, 5] per head) so softmax
     denominators accumulate in PSUM row 32c+4 for free; col tiling
     (tile_position=(0,32c)) packs 4 heads into one 2-bank PSUM accumulator
     (start=False onto DVE-memset PSUM). The chunk loop is software-
     pipelined one stage (scores+exp emitted before the previous chunk's
     AV) so the in-order PE queue never blocks the next exp. PSUM budget:
     2x3-bank score slots + 2-bank accumulator = all 8 banks.
  5. Tail (pipelined by query-half): one-hot const matmuls (fp32r) gather
     denominators (em) and reorder o rows (gm) into (h,d) order; DVE
     reciprocal + multiply normalizes; proj matmul; BN folded into one ACT
     Identity(scale,bias); unpool via masked multiplies (3 DVE + 1 GpSimd).
"""
import numpy as np

DIM = 64
HEAD_DIM = 4
NUM_HEADS = 16
B = 8
H = W = 64
HP = WP = 32
N = HP * WP          # 1024 tokens
NKT = 8              # key tiles of 128
BN_EPS = 1e-5

_CACHE = {}


def _build_program():
    import concourse.bass as bass
    import concourse.mybir as mybir
    import concourse.tile as tile
    from concourse import bacc

    f32 = mybir.dt.float32
    f32r = mybir.dt.float32r
    bf16 = mybir.dt.bfloat16
    AF = mybir.ActivationFunctionType
    OP = mybir.AluOpType

    nc = bacc.Bacc("TRN2", debug=False)

    x_d = nc.dram_tensor("x", [DIM, H * W], f32, kind="ExternalInput").ap()
    wa_d = nc.dram_tensor("wa", [128, 512], f32r, kind="ExternalInput").ap()
    wb_d = nc.dram_tensor("wb", [128, 1154], f32, kind="ExternalInput").ap()
    out_d = nc.dram_tensor("out", [DIM, H * W], f32, kind="ExternalOutput").ap()

    with tile.TileContext(nc) as tc:
        with (
            tc.tile_pool(name="singles", bufs=1) as sg1,
            tc.tile_pool(name="expp", bufs=4) as expp,
        ):
            # hoist the ACT exp-table load to t=0 via a dummy exp
            warm = sg1.tile([1, 1], f32)
            nc.vector.memset(warm, 0.0)
            nc.scalar.activation(warm, warm, AF.Exp)

            # ---------- loads (2 packed weight DMAs + x in 2 halves) ----------
            x_sb = sg1.tile([DIM, H * W], f32)
            xr = x_sb.rearrange("p (i ti j tj) -> p i ti j tj", ti=2, tj=2, j=WP)
            for qq in range(4):
                nc.sync.dma_start(out=x_sb[:, qq * 1024:(qq + 1) * 1024],
                                  in_=x_d[:, qq * 1024:(qq + 1) * 1024])
            wa_sb = sg1.tile([128, 512], f32r)
            nc.sync.dma_start(out=wa_sb, in_=wa_d)
            wb_sb = sg1.tile([128, 1154], f32)
            nc.sync.dma_start(out=wb_sb, in_=wb_d)
            em_sb = [wa_sb[:, 64 * sg:64 * sg + 64] for sg in range(4)]
            gm_sb = [wa_sb[:, 256 + 64 * sg:256 + 64 * sg + 64] for sg in range(4)]
            wqp_sb = [wb_sb[0:64, 128 * sg:128 * sg + 128] for sg in range(4)]
            wkp2_sb = [wb_sb[64:128, 512 + 128 * sg:512 + 128 * sg + 128]
                       for sg in range(4)]
            wv_sb = wb_sb[0:64, 1024:1088]
            wv2_sb = wb_sb[64:128, 1024:1088]
            wproj_sb = wb_sb[0:64, 1088:1152]
            bns_sb = wb_sb[0:64, 1152:1153]
            bnb_sb = wb_sb[0:64, 1153:1154]

            # ---------- maxpool (per x-half) ----------
            m01 = sg1.tile([DIM, N], f32)
            m23 = sg1.tile([DIM, N], f32)
            pooled = sg1.tile([DIM, N], f32)
            m01r = m01.rearrange("p (i j) -> p i j", j=WP)
            m23r = m23.rearrange("p (i j) -> p i j", j=WP)
            pooledr = pooled.rearrange("p (i j) -> p i j", j=WP)
            v = [xr[:, :, 0, :, 0], xr[:, :, 0, :, 1],
                 xr[:, :, 1, :, 0], xr[:, :, 1, :, 1]]
            for hh in range(4):
                sl = slice(hh * 8, (hh + 1) * 8)
                nc.vector.tensor_tensor(m01r[:, sl], v[0][:, sl], v[1][:, sl], op=OP.max)
                nc.vector.tensor_tensor(m23r[:, sl], v[2][:, sl], v[3][:, sl], op=OP.max)
                nc.vector.tensor_tensor(pooledr[:, sl], m01r[:, sl], m23r[:, sl], op=OP.max)

            # ---------- qkv packs + V~ (per pooled-half) ----------
            ones16 = sg1.tile([128, 16], f32)
            nc.vector.memset(ones16, 1.0)
            qtp = [sg1.tile([128, N], f32r, tag=f"qtp{sg}", name=f"qtp{sg}") for sg in range(4)]
            ktp = [sg1.tile([128, N], f32r, tag=f"ktp{sg}", name=f"ktp{sg}") for sg in range(4)]
            vt = [sg1.tile([128, 16, 5], bf16, tag=f"vt{kt}", name=f"vt{kt}") for kt in range(NKT)]
            pooled2 = sg1.tile([128, N], f32)
            for qh in range(2):
                qsl = slice(qh * 512, (qh + 1) * 512)
                nc.vector.tensor_copy(pooled2[64:128, qsl], pooled[:, qsl])
            dummy_bf = sg1.tile([64, 512], bf16)
            nc.vector.memset(dummy_bf, 1.0)
            with (
                tc.tile_pool(name="prepq", bufs=6, space="PSUM") as prepq,
                tc.tile_pool(name="prepv", bufs=2, space="PSUM") as prepv,
            ):
                # PE warmup during the x DMA: back-to-back discarded matmuls
                # keep the HAM busy-window hot so prep matmuls run at 2.4GHz
                for wi in range(12):
                    w_ps = prepq.tile([128, 512], f32, tag="qkps")
                    nc.tensor.matmul(w_ps, dummy_bf[:, 0:128], dummy_bf,
                                     start=True, stop=True)
                for qh in range(2):
                    qsl = slice(qh * 512, (qh + 1) * 512)
                    for sg in range(4):
                        # q on row-strip 0 and k on strip 64 run concurrent;
                        # copies alternate ACT (early half) / DVE
                        cpq = nc.scalar.copy if qh == 0 else nc.vector.tensor_copy
                        cpk = nc.vector.tensor_copy
                        qt_ps = prepq.tile([128, 512], f32, tag="qkps")
                        nc.tensor.matmul(qt_ps, wqp_sb[sg], pooled[:, qsl],
                                         start=True, stop=True,
                                         tile_position=(0, 0))
                        kt_ps = prepq.tile([128, 512], f32, tag="qkps")
                        nc.tensor.matmul(kt_ps, wkp2_sb[sg], pooled2[64:128, qsl],
                                         start=True, stop=True,
                                         tile_position=(64, 0))
                        cpq(qtp[sg][:, qsl], qt_ps)
                        cpk(ktp[sg][:, qsl], kt_ps)
                    for kt in range(qh * 4, qh * 4 + 4):
                        v_ps = prepv.tile([128, DIM], f32, tag="vps")
                        if kt % 2 == 0:
                            nc.tensor.matmul(
                                v_ps, pooled[:, kt * 128:(kt + 1) * 128], wv_sb,
                                start=True, stop=True, tile_position=(0, 0))
                        else:
                            nc.tensor.matmul(
                                v_ps, pooled2[64:128, kt * 128:(kt + 1) * 128],
                                wv2_sb,
                                start=True, stop=True, tile_position=(64, 0))
                        nc.vector.tensor_copy(
                            vt[kt][:, :, 0:4],
                            v_ps.rearrange("p (h e) -> p h e", e=4))
                        nc.vector.tensor_copy(
                            vt[kt].rearrange("p h e -> p (h e)")[:, 4::5], ones16)

            # ---------- argmax masks (DVE, fills attention idle) ----------
            masks = []
            nf = None
            for p in range(4):
                eq = sg1.tile([DIM, N], f32, tag=f"eq{p}")
                eqr = eq.rearrange("p (i j) -> p i j", j=WP)
                nc.vector.tensor_tensor(eqr, v[p], pooledr, op=OP.is_equal)
                if p == 0:
                    masks.append(eq)
                    nf = sg1.tile([DIM, N], f32, tag="nf0")
                    nc.vector.tensor_scalar(nf, eq, -1.0, 1.0, op0=OP.mult, op1=OP.add)
                else:
                    mk = sg1.tile([DIM, N], f32, tag=f"mk{p}")
                    nc.vector.tensor_tensor(mk, eq, nf, op=OP.mult)
                    masks.append(mk)
                    if p < 3:
                        nf2 = sg1.tile([DIM, N], f32, tag=f"nf{p}")
                        nc.vector.tensor_tensor(nf2, nf, mk, op=OP.subtract)
                        nf = nf2

            # ---------- attention ----------
            chunks = [
                [(0, 0), (1, 0), (2, 0)],
                [(3, 0), (0, 1), (1, 1)],
                [(2, 1), (3, 1)],
            ]
            o_sb = [sg1.tile([128, N], f32r, tag=f"osb{sg}", name=f"osb{sg}") for sg in range(4)]
            with (
                tc.tile_pool(name="spsum", bufs=2, space="PSUM") as spsum,
                tc.tile_pool(name="opsum", bufs=1, space="PSUM") as opsum,
            ):
                for sg in range(4):
                    o_ps = opsum.tile([128, N], f32, tag="ops")
                    nc.vector.memset(o_ps, 0.0)
                    # software-pipeline by one chunk: emit scores+exp, then
                    # the PREVIOUS chunk's AV matmuls, so PE's in-order queue
                    # never parks AV work in front of the next chunk's scores
                    pend = None

                    def flush_av(pend):
                        kt0, ch0, e0 = pend
                        for i, (c, qh) in enumerate(ch0):
                            nc.tensor.matmul(
                                o_ps[32 * c:32 * c + 5,
                                     qh * 512:(qh + 1) * 512],
                                vt[kt0][:, 4 * sg + c, :],
                                e0[:, i * 512:(i + 1) * 512],
                                start=False, stop=(kt0 == NKT - 1),
                                skip_group_check=True,
                                tile_position=(0, 32 * c))

                    for kt in range(NKT):
                        for ch in chunks:
                            ncb = len(ch)
                            s_ps = spsum.tile([128, 3 * 512], f32, tag="slot")
                            for i, (c, qh) in enumerate(ch):
                                nc.tensor.matmul(
                                    s_ps[:, i * 512:(i + 1) * 512],
                                    ktp[sg][32 * c:32 * c + 4,
                                            kt * 128:(kt + 1) * 128],
                                    qtp[sg][32 * c:32 * c + 4,
                                            qh * 512:(qh + 1) * 512],
                                    start=True, stop=True,
                                    tile_position=(32 * c, 0))
                            e_sb = expp.tile([128, 3 * 512], bf16, tag="exp")
                            nc.scalar.activation(
                                e_sb[:, :ncb * 512], s_ps[:, :ncb * 512],
                                AF.Exp, scale=0.5)
                            if pend is not None:
                                flush_av(pend)
                            pend = (kt, ch, e_sb)
                    flush_av(pend)
                    # evict accumulator (o_ps fully initialized via memset)
                    nc.vector.tensor_copy(o_sb[sg][:, 0:512], o_ps[:, 0:512])
                    nc.vector.tensor_copy(o_sb[sg][:, 512:1024], o_ps[:, 512:1024])

            # ---------- tail: per-half pipeline normalize+proj+bn+unpool ----------
            out_sb = sg1.tile([DIM, H * W], f32)
            outr = out_sb.rearrange("p (i ti j tj) -> p i ti j tj",
                                    ti=2, tj=2, j=WP)
            yr_all = []
            with tc.tile_pool(name="tailps", bufs=1, space="PSUM") as tailps:
                dr = sg1.tile([DIM, N], f32)
                onorm = sg1.tile([DIM, N], f32)
                y = sg1.tile([DIM, N], f32)
                for qh in range(2):
                    qsl = slice(qh * 512, (qh + 1) * 512)
                    d_ps = tailps.tile([DIM, 512], f32, tag=f"dps{qh}")
                    o2_ps = tailps.tile([DIM, 512], f32, tag=f"o2ps{qh}")
                    for sg in range(4):
                        nc.tensor.matmul(
                            d_ps, em_sb[sg], o_sb[sg][:, qsl],
                            start=(sg == 0), stop=(sg == 3))
                    for sg in range(4):
                        nc.tensor.matmul(
                            o2_ps, gm_sb[sg], o_sb[sg][:, qsl],
                            start=(sg == 0), stop=(sg == 3))
                    nc.vector.reciprocal(dr[:, qsl], d_ps)
                    nc.vector.tensor_tensor(onorm[:, qsl], o2_ps, dr[:, qsl],
                                            op=OP.mult)
                    pj_ps = tailps.tile([DIM, 512], f32, tag=f"pjps{qh}")
                    nc.tensor.matmul(
                        pj_ps, wproj_sb, onorm[:, qsl],
                        start=True, stop=True)
                    nc.scalar.activation(
                        y[:, qsl], pj_ps, AF.Identity, bias=bnb_sb, scale=bns_sb)
                    yr = y.rearrange("p (i j) -> p i j", j=WP)
                    sl = slice(qh * 16, (qh + 1) * 16)
                    for p in range(4):
                        mr = masks[p].rearrange("p (i j) -> p i j", j=WP)
                        eng = nc.vector if p < 3 else nc.gpsimd
                        eng.tensor_tensor(
                            outr[:, sl, p // 2, :, p % 2], yr[:, sl], mr[:, sl],
                            op=OP.mult)
                    nc.sync.dma_start(
                        out=out_d[:, qh * 2048:(qh + 1) * 2048],
                        in_=out_sb[:, qh * 2048:(qh + 1) * 2048])

    nc.compile()
    return nc


def _host_inputs(x, w_qkv, w_proj, gamma, beta, bn_mean, bn_var):
    """Build the per-core input maps (host-side packing)."""
    wq = w_qkv[:, 0:64]
    wk = w_qkv[:, 64:128]
    wv = np.ascontiguousarray(w_qkv[:, 128:192], dtype=np.float32)
    wqp = np.zeros((4, DIM, 128), np.float32)
    wkp = np.zeros((4, DIM, 128), np.float32)
    em = np.zeros((4, 128, DIM), np.float32)
    gm = np.zeros((4, 128, DIM), np.float32)
    for sg in range(4):
        for c in range(4):
            h = 4 * sg + c
            for d in range(HEAD_DIM):
                wqp[sg][:, 32 * c + d] = wq[:, 4 * h + d]
                wkp[sg][:, 32 * c + d] = wk[:, 4 * h + d]
                gm[sg][32 * c + d, 4 * h + d] = 1.0
                em[sg][32 * c + 4, 4 * h + d] = 1.0
    inv = gamma / np.sqrt(bn_var + BN_EPS)
    bns = inv.reshape(DIM, 1).astype(np.float32)
    bnb = (beta - bn_mean * inv).reshape(DIM, 1).astype(np.float32)
    wproj = np.ascontiguousarray(w_proj, dtype=np.float32)

    wa = np.zeros((128, 512), np.float32)
    for sg in range(4):
        wa[:, 64 * sg:64 * sg + 64] = em[sg]
        wa[:, 256 + 64 * sg:256 + 64 * sg + 64] = gm[sg]
    wb = np.zeros((128, 1154), np.float32)
    for sg in range(4):
        wb[0:64, 128 * sg:128 * sg + 128] = wqp[sg]
        wb[0:64, 512 + 128 * sg:512 + 128 * sg + 128] = wkp[sg]
    wb[0:64, 1024:1088] = wv
    wb[0:64, 1088:1152] = wproj
    wb[0:64, 1152:1153] = bns
    wb[0:64, 1153:1154] = bnb
    wb[64:128, :] = wb[0:64, :]
    shared = {"wa": wa, "wb": wb}
    in_maps = []
    for b in range(B):
        m = dict(shared)
        m["x"] = np.ascontiguousarray(
            np.asarray(x)[b].reshape(DIM, H * W), dtype=np.float32)
        in_maps.append(m)
    return in_maps


def kernel(x, w_qkv, w_proj, gamma, beta, bn_mean, bn_var):
    from concourse import bass_utils

    if "nc" not in _CACHE:
        _CACHE["nc"] = _build_program()
    nc = _CACHE["nc"]
    in_maps = _host_inputs(
        np.asarray(x), np.asarray(w_qkv), np.asarray(w_proj),
        np.asarray(gamma), np.asarray(beta),
        np.asarray(bn_mean), np.asarray(bn_var))
    res = bass_utils.run_bass_kernel_spmd(nc, in_maps, core_ids=list(range(B)))
    out = np.stack([res.results[b]["out"].reshape(DIM, H, W) for b in range(B)])
    return out.astype(np.float32)




# revision 16
# speedup vs baseline: 1.6517x; 1.6517x over previous
"""Trainium2 Bass kernel for nn_D_GA_1812476199112 (maxpool -> 16-head
attention over 1024 tokens -> proj -> batchnorm -> maxunpool).

Sharding: data-parallel over batch B=8, one batch element per NeuronCore.
Everything is local per core; no collectives.

Per-core pipeline, v2 ("flip-AV + 2-engine exp"):
  * Pre/post stages run in a [128, 2048] partition-packed layout
    (partition = channel + 64*image-half), halving the per-op free sizes
    of maxpool / argmax masks / unpool and the x/out DMA times. All of
    them run on GPSIMD (which cannot touch PSUM), freeing ACT/DVE.
  * QKV prep matmuls run in f32r (1 cycle/row) with strip-packed
    host-permuted weights; Q/K packs are evicted PSUM->SBUF as paired
    [128, 1024] copies alternating ACT/DVE.
  * Scores S^T [128 keys, 1024 queries] per (head, kt) chunk (two 512-q
    matmuls into one 2-bank PSUM slot, 3 slots); exp is split across ACT
    (table exp, bf16 out) and DVE (Schraudolph bit-trick: int16
    y = round(a*s + b) reinterpreted as bf16 ~ exp(s/2), ~3% elementwise
    error that largely cancels in the softmax ratio; ~1e-3 end-to-end).
  * AV is "flipped": each E block [128k, 128q] is the stationary operand
    (weight load is free) against a ones-augmented V~ [128k, 5] moving
    operand, producing o^T [128q, 5] in 5 cycles/matmul, accumulated over
    key tiles into a memset 2-bank PSUM accumulator with start=False
    (a start=True would mark the whole 2KB zero-region pending-zero and
    wipe sibling regions); denominators ride along in column 4.
  * Tail: strided DVE reciprocal+normalize in [q, (h,d)] layout, identity
    matmul transpose back to [(h,d), q], f32r proj, fused BN via ACT
    Identity(scale,bias), unpool via masked multiplies on GPSIMD.
"""
import numpy as np

DIM = 64
HEAD_DIM = 4
NUM_HEADS = 16
B = 8
H = W = 64
HP = WP = 32
N = HP * WP          # 1024 tokens
NKT = 8              # key tiles of 128
BN_EPS = 1e-5

# Schraudolph constants for exp(0.5*s) in bf16-bits:
#   int16 y = round(A_SCH * s + B_SCH); reinterpret(y) ~ exp(0.5*s)
A_SCH = 0.5 * 128.0 / float(np.log(2.0))
B_SCH = 127.0 * 128.0 - 5.5

# wb column layout
_WQP0 = 0              # 4 x [128, 128] strip-packed q weights (doubled rows)
_WKP0 = 512            # 4 x [128, 128] strip-packed k weights
_WV0 = 1024            # [128, 64] v weights (doubled rows)
_WPJ0 = 1088           # [64, 64] proj weights (rows 0..64)
_I0 = 1152             # [128, 128] identity
_BNS = 1280            # [128, 1] bn scale (doubled)
_BNB = 1281            # [128, 1] bn bias (doubled)
_WBC = 1282

_CACHE = {}


def _region_col(m):
    """PSUM accumulator column for region m = 8*head + qtile (5 cols each,
    split so no [*,5] block crosses the 512-col PSUM bank boundary)."""
    return 5 * m if m < 102 else 512 + 5 * (m - 102)


def _exp_schedule():
    """Greedy assignment of the 128 exp chunks to ACT/DVE by projected
    finish time. Offsets model each engine's upfront duties."""
    ready = {"act": 6000.0, "dve": 8000.0}
    cost = {"act": 1038.0, "dve": 1192.0}
    out = []
    t_pe = 9000.0
    for _ in range(128):
        t_pe += 427.0
        e = min(ready, key=lambda k: max(ready[k], t_pe) + cost[k])
        out.append(e)
        ready[e] = max(ready[e], t_pe) + cost[e]
    return out


def _build_program():
    import concourse.mybir as mybir
    import concourse.tile as tile
    from concourse import bacc

    f32 = mybir.dt.float32
    f32r = mybir.dt.float32r
    bf16 = mybir.dt.bfloat16
    i16 = mybir.dt.int16
    AF = mybir.ActivationFunctionType
    OP = mybir.AluOpType

    nc = bacc.Bacc("TRN2", debug=False)

    x_d = nc.dram_tensor("x", [DIM, H * W], f32, kind="ExternalInput").ap()
    wb_d = nc.dram_tensor("wb", [128, _WBC], f32r, kind="ExternalInput").ap()
    out_d = nc.dram_tensor("out", [DIM, H * W], f32, kind="ExternalOutput").ap()

    with tile.TileContext(nc) as tc:
        with (
            tc.tile_pool(name="singles", bufs=1) as sg1,
            tc.tile_pool(name="expp", bufs=6) as expp,
            tc.tile_pool(name="spsum", bufs=3, space="PSUM") as spsum,
            tc.tile_pool(name="opsum", bufs=1, space="PSUM") as opsum,
        ):
            # hoist the ACT exp-table load to t=0 via a dummy exp
            warm = sg1.tile([1, 1], f32)
            nc.vector.memset(warm, 0.0)
            nc.scalar.activation(warm, warm, AF.Exp)

            # ---------- input DMAs ----------
            x_sb = sg1.tile([128, H * W // 2], f32)
            nc.sync.dma_start(out=x_sb[0:64, :], in_=x_d[:, 0:2048])
            nc.sync.dma_start(out=x_sb[64:128, :], in_=x_d[:, 2048:4096])
            wb_sb = sg1.tile([128, _WBC], f32r)
            nc.sync.dma_start(out=wb_sb, in_=wb_d)
            wqp = [wb_sb[:, _WQP0 + 128 * s:_WQP0 + 128 * s + 128] for s in range(4)]
            wkp = [wb_sb[:, _WKP0 + 128 * s:_WKP0 + 128 * s + 128] for s in range(4)]
            wv_sb = wb_sb[:, _WV0:_WV0 + 64]
            wpj_sb = wb_sb[:, _WPJ0:_WPJ0 + 64]
            i128_sb = wb_sb[:, _I0:_I0 + 128]
            bns_sb = wb_sb[:, _BNS:_BNS + 1].bitcast(f32)
            bnb_sb = wb_sb[:, _BNB:_BNB + 1].bitcast(f32)

            # ---------- PE warmup (p-state ramp) ----------
            dummy_bf = sg1.tile([64, 512], bf16)
            nc.vector.memset(dummy_bf, 1.0)
            for _ in range(8):
                w_ps = spsum.tile([128, 1024], f32, tag="slot")
                nc.tensor.matmul(w_ps[:, 0:512], dummy_bf[:, 0:128], dummy_bf,
                                 start=True, stop=True)

            # ---------- maxpool (DVE max; GPSIMD lacks max/is_equal) ----------
            # x_sb per-partition layout: 32 h-rows x 64 w; window elems:
            xr = x_sb.rearrange("p (i ti j tj) -> p i ti j tj", ti=2, tj=2, j=WP)
            v = [xr[:, :, 0, :, 0], xr[:, :, 0, :, 1],
                 xr[:, :, 1, :, 0], xr[:, :, 1, :, 1]]
            pooled = sg1.tile([128, N // 2], f32)
            m01 = sg1.tile([128, N // 2], f32)
            m23 = sg1.tile([128, N // 2], f32)
            pooledr = pooled.rearrange("p (i j) -> p i j", j=WP)
            m01r = m01.rearrange("p (i j) -> p i j", j=WP)
            m23r = m23.rearrange("p (i j) -> p i j", j=WP)
            nc.vector.tensor_tensor(m01r, v[0], v[1], op=OP.max)
            nc.vector.tensor_tensor(m23r, v[2], v[3], op=OP.max)
            nc.vector.tensor_tensor(pooledr, m01r, m23r, op=OP.max)
            # f32r copy for the matmul side (exact pooled stays for masks)
            pooled2 = sg1.tile([128, N // 2], f32r)
            nc.gpsimd.tensor_copy(pooled2, pooled)

            # ---------- QKV prep (f32r matmuls; paired ACT/DVE evictions) ----
            qtp = [sg1.tile([128, N], f32r, tag=f"qtp{s}", name=f"qtp{s}")
                   for s in range(4)]
            ktp = [sg1.tile([128, N], f32r, tag=f"ktp{s}", name=f"ktp{s}")
                   for s in range(4)]
            pooled_r = pooled2[:, :]
            evict = [nc.scalar.copy, nc.vector.tensor_copy]
            ev = 0
            for s in range(4):
                for w_sb, pack in ((wkp[s], ktp[s]), (wqp[s], qtp[s])):
                    p_ps = spsum.tile([128, 1024], f32, tag="slot")
                    for h in range(2):
                        psl = slice(64 * h, 64 * h + 64)
                        nc.tensor.matmul(p_ps[:, 512 * h:512 * h + 512],
                                         w_sb[psl],
                                         pooled_r[psl], start=True, stop=True)
                    evict[ev % 2](pack[:, :], p_ps)
                    ev += 1

            # V~ tiles [128 keys, 16 heads * (4 dims + ones)] in bf16
            vt = [sg1.tile([128, 80], bf16, tag=f"vt{kt}", name=f"vt{kt}")
                  for kt in range(NKT)]
            for kt in range(NKT):
                nc.gpsimd.memset(vt[kt], 1.0)
            for kt in range(NKT):
                h = kt // 4
                psl = slice(64 * h, 64 * h + 64)
                v_ps = spsum.tile([128, 1024], f32, tag="slot")
                nc.tensor.matmul(
                    v_ps[:, 0:64],
                    pooled_r[psl, 128 * (kt % 4):128 * (kt % 4) + 128],
                    wv_sb[psl], start=True, stop=True)
                nc.vector.tensor_copy(
                    vt[kt].rearrange("p (h e) -> p h e", e=5)[:, :, 0:4],
                    v_ps[:, 0:64].rearrange("p (h e) -> p h e", e=4))

            # ---------- argmax masks (GPSIMD; first-match semantics) ----------
            masks = []
            nf = None
            for p in range(4):
                eq = sg1.tile([128, N // 2], f32, tag=f"eq{p}")
                eqr = eq.rearrange("p (i j) -> p i j", j=WP)
                nc.vector.tensor_tensor(eqr, v[p], pooledr, op=OP.is_equal)
                if p == 0:
                    masks.append(eq)
                    nf = sg1.tile([128, N // 2], f32, tag="nf0")
                    nc.gpsimd.tensor_scalar(nf, eq, -1.0, 1.0,
                                            op0=OP.mult, op1=OP.add)
                else:
                    mk = sg1.tile([128, N // 2], f32, tag=f"mk{p}")
                    nc.gpsimd.tensor_tensor(mk, eq, nf, op=OP.mult)
                    masks.append(mk)
                    if p < 3:
                        nf2 = sg1.tile([128, N // 2], f32, tag=f"nf{p}")
                        nc.gpsimd.tensor_tensor(nf2, nf, mk, op=OP.subtract)
                        nf = nf2

            # ---------- attention ----------
            # Bank-init trick: one start=True matmul into each bank's pad
            # columns marks the whole 2KB zero-region pending-zero, so every
            # region's first start=False AV write initializes (no memset).
            # More than one start=True per bank would wipe sibling regions.
            o_acc = opsum.tile([128, 1024], f32)
            for bank_pad in (510, 1022):
                nc.tensor.matmul(o_acc[:, bank_pad:bank_pad + 2],
                                 dummy_bf[:, 0:128], dummy_bf[:, 0:2],
                                 start=True, stop=True,
                                 skip_group_check=True)
            assign = _exp_schedule()
            pend = []
            LAG = 3

            def flush_one():
                e_t, sg, c, kt = pend.pop(0)
                head = 4 * sg + c
                for J in range(8):
                    col = _region_col(16 * J + head)
                    nc.tensor.matmul(
                        o_acc[:, col:col + 5],
                        e_t[:, 128 * J:128 * J + 128].bitcast(bf16),
                        vt[kt][:, 5 * head:5 * head + 5],
                        start=False, stop=(kt == NKT - 1),
                        skip_group_check=True)

            ci = 0
            for sg in range(4):
                for kt in range(NKT):
                    for c in range(4):
                        s_ps = spsum.tile([128, 1024], f32, tag="slot")
                        for qh in range(2):
                            nc.tensor.matmul(
                                s_ps[:, 512 * qh:512 * qh + 512],
                                ktp[sg][32 * c:32 * c + 4,
                                        128 * kt:128 * kt + 128],
                                qtp[sg][32 * c:32 * c + 4,
                                        512 * qh:512 * qh + 512],
                                start=True, stop=True,
                                tile_position=(32 * c, 0))
                        e_t = expp.tile([128, 1024], i16, tag="exp")
                        eng = assign[ci]
                        ci += 1
                        if eng == "act":
                            nc.scalar.activation(
                                e_t[:, :].bitcast(bf16), s_ps,
                                AF.Exp, scale=0.5)
                        else:
                            nc.vector.tensor_scalar(
                                e_t, s_ps, A_SCH, B_SCH,
                                op0=OP.mult, op1=OP.add)
                        pend.append((e_t, sg, c, kt))
                        if len(pend) > LAG:
                            flush_one()
            while pend:
                flush_one()

            # ---------- tail ----------
            # denominators: region m col +4, strided over the 5-col regions
            o5a = o_acc[:, 0:510].rearrange("p (m e) -> p m e", e=5)
            o5b = o_acc[:, 512:642].rearrange("p (m e) -> p m e", e=5)
            rall = sg1.tile([128, 128], f32)
            nc.vector.reciprocal(rall[:, 0:102], o5a[:, :, 4])
            nc.vector.reciprocal(rall[:, 102:128], o5b[:, :, 4])
            rep = sg1.tile([128, 512], f32)
            rep4 = rep.rearrange("p (m e) -> p m e", e=4)
            for d in range(4):
                nc.vector.tensor_copy(rep4[:, :, d], rall)
            eno = sg1.tile([128, 512], f32r)
            eno4 = eno.rearrange("p (m e) -> p m e", e=4)
            nc.vector.tensor_tensor(eno4[:, 0:102], o5a[:, :, 0:4],
                                    rep4[:, 0:102], op=OP.mult)
            nc.vector.tensor_tensor(eno4[:, 102:128], o5b[:, :, 0:4],
                                    rep4[:, 102:128], op=OP.mult)
            # transpose back to [(h,d), q] via identity matmuls; eno cols are
            # (J, h, d)-contiguous under the J-major region map m = 16J+head
            ot_sb = sg1.tile([64, N], f32r)
            for jg in range(2):
                ot_ps = spsum.tile([128, 1024], f32, tag="slot")
                for jj in range(4):
                    J = 4 * jg + jj
                    nc.tensor.matmul(
                        ot_ps[0:64, 128 * jj:128 * jj + 128],
                        eno[:, 64 * J:64 * J + 64],
                        i128_sb,
                        start=True, stop=True)
                nc.scalar.copy(ot_sb[:, 512 * jg:512 * jg + 512],
                               ot_ps[0:64, 0:512])
            # proj + BN (per image-half) + unpool
            y128 = sg1.tile([128, 512], f32)
            out_sb = sg1.tile([128, H * W // 2], f32)
            outr = out_sb.rearrange("p (i ti j tj) -> p i ti j tj",
                                    ti=2, tj=2, j=WP)
            yr = y128.rearrange("p (i j) -> p i j", j=WP)
            for h in range(2):
                pj_ps = spsum.tile([128, 1024], f32, tag="slot")
                nc.tensor.matmul(
                    pj_ps[0:64, 0:512], wpj_sb[0:64],
                    ot_sb[:, 512 * h:512 * h + 512],
                    start=True, stop=True)
                psl = slice(64 * h, 64 * h + 64)
                nc.scalar.activation(y128[psl], pj_ps[0:64, 0:512], AF.Identity,
                                     bias=bnb_sb[psl], scale=bns_sb[psl])
                for p in range(4):
                    mr = masks[p].rearrange("p (i j) -> p i j", j=WP)
                    eng = nc.vector if p < 2 else nc.gpsimd
                    eng.tensor_tensor(
                        outr[psl, :, p // 2, :, p % 2], yr[psl], mr[psl],
                        op=OP.mult)
                nc.sync.dma_start(
                    out=out_d[:, 2048 * h:2048 * h + 2048],
                    in_=out_sb[64 * h:64 * h + 64, :])

    nc.compile()
    return nc


def _host_inputs(x, w_qkv, w_proj, gamma, beta, bn_mean, bn_var):
    """Build the per-core input maps (host-side packing)."""
    wq = w_qkv[:, 0:64]
    wk = w_qkv[:, 64:128]
    wv = np.ascontiguousarray(w_qkv[:, 128:192], dtype=np.float32)
    wb = np.zeros((128, _WBC), np.float32)
    for sg in range(4):
        for c in range(4):
            h = 4 * sg + c
            for d in range(HEAD_DIM):
                wb[0:64, _WQP0 + 128 * sg + 32 * c + d] = wq[:, 4 * h + d]
                wb[0:64, _WKP0 + 128 * sg + 32 * c + d] = wk[:, 4 * h + d]
    wb[0:64, _WV0:_WV0 + 64] = wv
    wb[0:64, _WPJ0:_WPJ0 + 64] = np.asarray(w_proj, dtype=np.float32)
    wb[0:64, _I0:_I0 + 128] = np.eye(64, 128, dtype=np.float32)
    wb[64:128, _I0:_I0 + 128] = np.eye(64, 128, 64, dtype=np.float32)
    inv = gamma / np.sqrt(bn_var + BN_EPS)
    wb[0:64, _BNS] = inv
    wb[0:64, _BNB] = beta - bn_mean * inv
    # double the 64-row blocks onto partitions 64..128 (identity handled above)
    wb[64:128, 0:_I0] = wb[0:64, 0:_I0]
    wb[64:128, _BNS:] = wb[0:64, _BNS:]

    shared = {"wb": wb}
    in_maps = []
    for b in range(B):
        m = dict(shared)
        m["x"] = np.ascontiguousarray(
            np.asarray(x)[b].reshape(DIM, H * W), dtype=np.float32)
        in_maps.append(m)
    return in_maps


def kernel(x, w_qkv, w_proj, gamma, beta, bn_mean, bn_var):
    from concourse import bass_utils

    if "nc" not in _CACHE:
        _CACHE["nc"] = _build_program()
    nc = _CACHE["nc"]
    in_maps = _host_inputs(
        np.asarray(x), np.asarray(w_qkv), np.asarray(w_proj),
        np.asarray(gamma), np.asarray(beta),
        np.asarray(bn_mean), np.asarray(bn_var))
    res = bass_utils.run_bass_kernel_spmd(nc, in_maps, core_ids=list(range(B)))
    out = np.stack([res.results[b]["out"].reshape(DIM, H, W) for b in range(B)])
    return out.astype(np.float32)
